# revision 15
# baseline (speedup 1.0000x reference)
"""Trainium2 Bass kernel for DoubleBinaryLinear:
    y = ((x * s0) @ B.T * s2) @ A.T * s4 + bias
with x [4, 2048, 4096] fp32 and binary (+-1) B, A [4096, 4096].

Strategy
--------
Data-parallel over tokens: 8192 tokens split 1024 per NeuronCore (8 cores),
each core runs the full two-layer pipeline on its slice. No collectives.
Weights replicated in fp8 e4m3 (exact for +-1 entries).

All matmuls use fp8 e4m3 with MatmulPerfMode.DoubleRow: each instruction
contracts TWO 128-row k-tiles (lhsT [128,2,128], rhs [128,2,512]) in the
same ~216ns a bf16 matmul needs for one k-tile — 2x effective throughput.

Activations use a magnitude-aware fp8 hi/lo split. The contraction dim of
each stage is permuted (host-side) by descending diagonal scaling (s0 for
stage 1, s2 for stage 3); the error contribution of column k scales with
its scaling value, so only the top N1 (N3) of 16 k-pairs get a correction
lo pass (lo = rne(v - hi), ~8 mantissa bits combined); the low-scaling
tail runs hi-only (~4 bits, but tiny contribution). N1 = N3 = 8 measures
rel_err 1.43e-2 vs the 2e-2 gate on the fixed seed-0 inputs, saving 25%
of matmul issue slots vs a full hi/lo split:

  stage 0: z = xT * s0 (sorted), split into fp8 hi (+ lo for top pairs)
  stage 1: h1 = B @ z     -- PSUM fp32, (16 + N1) DR matmuls per bank
  stage 2: h1s = h1 * (s2/4) (sorted), split hi (+ lo for top pairs);
           s2/4 keeps |h1s| well under the e4m3 max of 240 (4x folded
           into s4, exact)
  stage 3: h2 = A @ h1s   -- (16 + N3) DR matmuls per bank
  stage 4: yT = h2 * (4*s4) + bias   (output order unpermuted)
"""

import os

import numpy as np
import ml_dtypes

import concourse.bacc as bacc
import concourse.mybir as mybir
from concourse import tile
from concourse import bass_utils

P = 128
F32 = mybir.dt.float32
F16 = mybir.dt.float16
FP8 = mybir.dt.float8e4
E4M3 = ml_dtypes.float8_e4m3

IN_D = 4096
MID_D = 4096
OUT_D = 4096
BATCH = 4
SEQ = 2048
N_CORES = 8
T_CORE = (BATCH * SEQ) // N_CORES   # 1024 tokens per core
TC = 512                            # token chunk = matmul moving free dim
MG = 4                              # m-tiles per PSUM group
SDOWN = 0.25                        # mid-activation downscale (exact pow2)

KP1 = IN_D // (2 * P)               # 16 k-pairs, stage 1
KP3 = MID_D // (2 * P)              # 16 k-pairs, stage 3
N1 = 8                              # stage-1 k-pairs with lo pass
N3 = 8                              # stage-3 k-pairs with lo pass


def _build_nc():
    nI, nM, nO, nC = IN_D // P, MID_D // P, OUT_D // P, T_CORE // TC

    nc = bacc.Bacc(None, target_bir_lowering=False)
    xT = nc.dram_tensor("xT", [IN_D // 2, 2, T_CORE], F16,
                        kind="ExternalInput")
    # DoubleRow pair layouts: rows = kp*128+p, cols = mg*(2*MG*P) + kk*MG*P + m
    Bd = nc.dram_tensor("Bd", [IN_D // 2, 2 * MID_D], FP8, kind="ExternalInput")
    Ad = nc.dram_tensor("Ad", [MID_D // 2, 2 * OUT_D], FP8, kind="ExternalInput")
    nSC = (IN_D + MID_D + 2 * OUT_D) // P
    scd = nc.dram_tensor("sc", [P, nSC], F32, kind="ExternalInput")
    yT = nc.dram_tensor("yT", [OUT_D, T_CORE], F32, kind="ExternalOutput")

    mult = mybir.AluOpType.mult
    sub = mybir.AluOpType.subtract
    add = mybir.AluOpType.add
    dr = mybir.MatmulPerfMode.DoubleRow
    WG = 2 * MG * P                 # weight-tile free size (pair layout)

    with tile.TileContext(nc) as tc:
        with (
            tc.tile_pool(name="consts", bufs=1) as cpool,
            tc.tile_pool(name="zbuf", bufs=1) as zpool,
            tc.tile_pool(name="h1buf", bufs=1) as hpool,
            tc.tile_pool(name="xin", bufs=16) as xpool,
            tc.tile_pool(name="wts", bufs=24) as wpool,
            tc.tile_pool(name="yout", bufs=3) as ypool,
            tc.tile_pool(name="psum", bufs=8, space="PSUM") as pspool,
        ):
            sc_t = cpool.tile([P, nSC], F32, tag="sc")
            nc.sync.dma_start(sc_t[:], scd[:, :])
            s0_t = sc_t[:, 0:nI]
            s2_t = sc_t[:, nI:nI + nM]
            s4_t = sc_t[:, nI + nM:nI + nM + nO]
            bi_t = sc_t[:, nI + nM + nO:nSC]

            for c in range(nC):
                t0 = c * TC
                # stage 0: load x (one DMA per k-pair, pair layout), scale by
                # s0, split to fp8 hi(/lo). Chunk 0 interleaves the x stream
                # with a preload of the mg=0 weights on the opposite queue so
                # the PE can start as soon as pair 0 lands.
                wb_pre = None
                if c == 0:
                    wb_pre = []
                zhi = [zpool.tile([P, 2, TC], FP8, tag=f"zhi{k}", name=f"zhi{k}")
                       for k in range(KP1)]
                zlo = [zpool.tile([P, 2, TC], FP8, tag=f"zlo{k}", name=f"zlo{k}")
                       for k in range(N1)]
                for kp in range(KP1):
                    xt = xpool.tile([P, 2, TC], F16, tag="xt")
                    if c == 0:
                        xq = nc.sync if kp % 2 == 0 else nc.gpsimd
                        wq = nc.gpsimd if kp % 2 == 0 else nc.sync
                    else:
                        xq = nc.scalar
                        wq = None
                    xq.dma_start(xt[:], xT[kp * P:(kp + 1) * P, :, t0:t0 + TC])
                    if wb_pre is not None:
                        wt = wpool.tile([P, 2, MG * P], FP8, tag="wb",
                                        name=f"wbp{kp}")
                        wq.dma_start(wt[:], Bd[kp * P:(kp + 1) * P, 0:WG])
                        wb_pre.append(wt)
                    for kk in range(2):
                        i = 2 * kp + kk
                        if c == 0 and (kp == 0 or (kp >= N1 and kk == 1)):
                            nc.vector.tensor_scalar_mul(
                                zhi[kp][:, kk, :], xt[:, kk, :],
                                s0_t[:, i:i + 1])
                        else:
                            nc.scalar.activation(
                                zhi[kp][:, kk, :], xt[:, kk, :],
                                mybir.ActivationFunctionType.Copy,
                                scale=s0_t[:, i:i + 1])
                        if kp < N1:
                            nc.vector.scalar_tensor_tensor(
                                zlo[kp][:, kk, :], xt[:, kk, :],
                                s0_t[:, i:i + 1], zhi[kp][:, kk, :],
                                mult, sub)

                # stage 1: h1 = B @ z; stage 2: scale by s2/4, split
                h1hi = [hpool.tile([P, 2, TC], FP8, tag=f"h1hi{k}",
                                   name=f"h1hi{k}") for k in range(KP3)]
                h1lo = [hpool.tile([P, 2, TC], FP8, tag=f"h1lo{k}",
                                   name=f"h1lo{k}") for k in range(N3)]
                for mg in range(nM // MG):
                    pss = [pspool.tile([P, TC], F32, tag="ps", name="ps")
                           for _ in range(MG)]
                    for kp in range(KP1):
                        if wb_pre is not None and mg == 0:
                            wt = wb_pre[kp]
                        else:
                            wt = wpool.tile([P, 2, MG * P], FP8, tag="wb")
                            q = nc.sync if kp % 2 == 0 else nc.gpsimd
                            q.dma_start(wt[:], Bd[kp * P:(kp + 1) * P,
                                                  mg * WG:(mg + 1) * WG])
                        hi_stop = kp == KP1 - 1 and N1 < KP1
                        lo_stop = kp == KP1 - 1 and N1 == KP1
                        for m_ in range(MG):
                            lhsT = wt[:, :, m_ * P:(m_ + 1) * P]
                            nc.tensor.matmul(pss[m_][:], lhsT, zhi[kp][:],
                                             start=(kp == 0), stop=hi_stop,
                                             perf_mode=dr)
                            if kp < N1:
                                nc.tensor.matmul(pss[m_][:], lhsT, zlo[kp][:],
                                                 start=False, stop=lo_stop,
                                                 perf_mode=dr)
                    for m_ in range(MG):
                        m = mg * MG + m_
                        mp, sl = m // 2, m % 2
                        nc.scalar.activation(
                            h1hi[mp][:, sl, :], pss[m_][:],
                            mybir.ActivationFunctionType.Copy,
                            scale=s2_t[:, m:m + 1])
                        if mp < N3:
                            nc.vector.scalar_tensor_tensor(
                                h1lo[mp][:, sl, :], pss[m_][:],
                                s2_t[:, m:m + 1], h1hi[mp][:, sl, :],
                                mult, sub)

                # stage 3: h2 = A @ h1s; stage 4: y = h2*s4 + bias
                for og in range(nO // MG):
                    pso = [pspool.tile([P, TC], F32, tag="ps", name="ps")
                           for _ in range(MG)]
                    for kp in range(KP3):
                        wt2 = wpool.tile([P, 2, MG * P], FP8, tag="wa")
                        q = nc.sync if kp % 2 == 0 else nc.gpsimd
                        q.dma_start(wt2[:], Ad[kp * P:(kp + 1) * P,
                                               og * WG:(og + 1) * WG])
                        hi_stop = kp == KP3 - 1 and N3 < KP3
                        lo_stop = kp == KP3 - 1 and N3 == KP3
                        for o_ in range(MG):
                            lhsT = wt2[:, :, o_ * P:(o_ + 1) * P]
                            nc.tensor.matmul(pso[o_][:], lhsT, h1hi[kp][:],
                                             start=(kp == 0), stop=hi_stop,
                                             perf_mode=dr)
                            if kp < N3:
                                nc.tensor.matmul(pso[o_][:], lhsT, h1lo[kp][:],
                                                 start=False, stop=lo_stop,
                                                 perf_mode=dr)
                    for o_ in range(MG):
                        o = og * MG + o_
                        yt = ypool.tile([P, TC], F32, tag="yt")
                        nc.vector.tensor_scalar(
                            yt[:], pso[o_][:], s4_t[:, o:o + 1],
                            bi_t[:, o:o + 1], mult, add)
                        yq = nc.scalar if o_ % 2 == 0 else nc.sync
                        yq.dma_start(yT[o * P:(o + 1) * P, t0:t0 + TC],
                                     yt[:])

    nc.compile()
    return nc


_NC_CACHE = None


def _get_nc():
    global _NC_CACHE
    if _NC_CACHE is None:
        _NC_CACHE = _build_nc()
    return _NC_CACHE


def _col_major(v):
    return np.ascontiguousarray(
        np.asarray(v, dtype=np.float32).reshape(-1, P).T)


def _pair_layout(WT, mg_tiles):
    """[K, M] -> [K/2, 2*M] DoubleRow layout: out[kp*128+p, mg*WG + kk*MG*128
    + m] = WT[kp*256 + kk*128 + p, mg*MG*128 + m], cast to fp8 e4m3."""
    K, M = WT.shape
    w = WT.reshape(K // 256, 2, P, mg_tiles, MG * P).transpose(0, 2, 3, 1, 4)
    return np.ascontiguousarray(w).astype(E4M3).reshape(K // 2, 2 * M)


def make_in_maps(x, scaling0, B, scaling2, A, scaling4, bias):
    x = np.asarray(x, dtype=np.float32)
    s0 = np.asarray(scaling0, dtype=np.float32)
    s2 = np.asarray(scaling2, dtype=np.float32)
    B = np.asarray(B, dtype=np.float32)
    A = np.asarray(A, dtype=np.float32)

    # magnitude-sorted contraction dims (descending scaling)
    p1 = np.argsort(-s0)
    p3 = np.argsort(-s2)

    xf = x.reshape(BATCH * SEQ, IN_D)[:, p1].astype(np.float16)
    Bd = _pair_layout(np.ascontiguousarray(B[np.ix_(p3, p1)].T),
                      MID_D // (MG * P))
    Ad = _pair_layout(np.ascontiguousarray(A[:, p3].T), OUT_D // (MG * P))
    sc = np.ascontiguousarray(np.concatenate(
        [_col_major(v) for v in
         (s0[p1], s2[p3] * SDOWN,
          np.asarray(scaling4, dtype=np.float32) / SDOWN, bias)], axis=1))

    in_maps = []
    for c in range(N_CORES):
        xs = xf[c * T_CORE:(c + 1) * T_CORE]
        # [t, kp, kk, p] -> [kp, p, kk, t]
        xsw = np.ascontiguousarray(
            xs.reshape(T_CORE, KP1, 2, P).transpose(1, 3, 2, 0))
        in_maps.append({
            "xT": xsw.reshape(IN_D // 2, 2, T_CORE),
            "Bd": Bd, "Ad": Ad, "sc": sc,
        })
    return in_maps


def kernel(x, scaling0, B, scaling2, A, scaling4, bias):
    # The profile hook isn't available in every environment; force the
    # plain execution path.
    os.environ.setdefault("BASS_NEVER_TRACE", "1")

    in_maps = make_in_maps(x, scaling0, B, scaling2, A, scaling4, bias)
    nc = _get_nc()
    res = bass_utils.run_bass_kernel_spmd(
        nc, in_maps, core_ids=list(range(N_CORES)))

    y = np.empty((BATCH * SEQ, OUT_D), dtype=np.float32)
    for c in range(N_CORES):
        y[c * T_CORE:(c + 1) * T_CORE] = res.results[c]["yT"].T
    return y.reshape(BATCH, SEQ, OUT_D)


# revision 16
# speedup vs baseline: 1.0054x; 1.0054x over previous
"""Trainium2 Bass kernel for DoubleBinaryLinear:
    y = ((x * s0) @ B.T * s2) @ A.T * s4 + bias
with x [4, 2048, 4096] fp32 and binary (+-1) B, A [4096, 4096].

Strategy
--------
Data-parallel over tokens: 8192 tokens split 1024 per NeuronCore (8 cores),
each core runs the full two-layer pipeline on its slice. No collectives.
Weights replicated in fp8 e4m3 (exact for +-1 entries).

All matmuls use fp8 e4m3 with MatmulPerfMode.DoubleRow: each instruction
contracts TWO 128-row k-tiles (lhsT [128,2,128], rhs [128,2,512]) in the
same ~216ns a bf16 matmul needs for one k-tile — 2x effective throughput.

Activations use a magnitude-aware fp8 hi/lo split. The contraction dim of
each stage is permuted (host-side) by descending diagonal scaling (s0 for
stage 1, s2 for stage 3); the error contribution of column k scales with
its scaling value, so only the top N1 (N3) of 16 k-pairs get a correction
lo pass (lo = rne(v - hi), ~8 mantissa bits combined); the low-scaling
tail runs hi-only (~4 bits, but tiny contribution). N1 = N3 = 8 measures
rel_err 1.43e-2 vs the 2e-2 gate on the fixed seed-0 inputs, saving 25%
of matmul issue slots vs a full hi/lo split:

  stage 0: z = xT * s0 (sorted), split into fp8 hi (+ lo for top pairs)
  stage 1: h1 = B @ z     -- PSUM fp32, (16 + N1) DR matmuls per bank
  stage 2: h1s = h1 * (s2/4) (sorted), split hi (+ lo for top pairs);
           s2/4 keeps |h1s| well under the e4m3 max of 240 (4x folded
           into s4, exact)
  stage 3: h2 = A @ h1s   -- (16 + N3) DR matmuls per bank
  stage 4: yT = h2 * (4*s4) + bias   (output order unpermuted)
"""

import os

import numpy as np
import ml_dtypes

import concourse.bacc as bacc
import concourse.mybir as mybir
from concourse import tile
from concourse import bass_utils

P = 128
F32 = mybir.dt.float32
F16 = mybir.dt.float16
FP8 = mybir.dt.float8e4
E4M3 = ml_dtypes.float8_e4m3

IN_D = 4096
MID_D = 4096
OUT_D = 4096
BATCH = 4
SEQ = 2048
N_CORES = 8
T_CORE = (BATCH * SEQ) // N_CORES   # 1024 tokens per core
TC = 512                            # token chunk = matmul moving free dim
MG = 4                              # m-tiles per PSUM group
SDOWN = 0.25                        # mid-activation downscale (exact pow2)

KP1 = IN_D // (2 * P)               # 16 k-pairs, stage 1
KP3 = MID_D // (2 * P)              # 16 k-pairs, stage 3
N1 = 8                              # stage-1 k-pairs with lo pass
N3 = 8                              # stage-3 k-pairs with lo pass


def _build_nc():
    nI, nM, nO, nC = IN_D // P, MID_D // P, OUT_D // P, T_CORE // TC

    nc = bacc.Bacc(None, target_bir_lowering=False)
    xT = nc.dram_tensor("xT", [IN_D // 2, 2, T_CORE], F16,
                        kind="ExternalInput")
    # DoubleRow pair layouts: rows = kp*128+p, cols = mg*(2*MG*P) + kk*MG*P + m
    Bd = nc.dram_tensor("Bd", [IN_D // 2, 2 * MID_D], FP8, kind="ExternalInput")
    Ad = nc.dram_tensor("Ad", [MID_D // 2, 2 * OUT_D], FP8, kind="ExternalInput")
    nSC = (IN_D + MID_D + 2 * OUT_D) // P
    scd = nc.dram_tensor("sc", [P, nSC], F32, kind="ExternalInput")
    yT = nc.dram_tensor("yT", [OUT_D, T_CORE], F32, kind="ExternalOutput")

    mult = mybir.AluOpType.mult
    sub = mybir.AluOpType.subtract
    add = mybir.AluOpType.add
    dr = mybir.MatmulPerfMode.DoubleRow
    WG = 2 * MG * P                 # weight-tile free size (pair layout)

    with tile.TileContext(nc) as tc:
        with (
            tc.tile_pool(name="consts", bufs=1) as cpool,
            tc.tile_pool(name="zbuf", bufs=1) as zpool,
            tc.tile_pool(name="h1buf", bufs=1) as hpool,
            tc.tile_pool(name="xin", bufs=16) as xpool,
            tc.tile_pool(name="wts", bufs=24) as wpool,
            tc.tile_pool(name="yout", bufs=3) as ypool,
            tc.tile_pool(name="psum", bufs=8, space="PSUM") as pspool,
        ):
            sc_t = cpool.tile([P, nSC], F32, tag="sc")
            nc.sync.dma_start(sc_t[:], scd[:, :])
            s0_t = sc_t[:, 0:nI]
            s2_t = sc_t[:, nI:nI + nM]
            s4_t = sc_t[:, nI + nM:nI + nM + nO]
            bi_t = sc_t[:, nI + nM + nO:nSC]

            for c in range(nC):
                t0 = c * TC
                # stage 0: load x (one DMA per k-pair, pair layout), scale by
                # s0, split to fp8 hi(/lo). Chunk 0 interleaves the x stream
                # with a preload of the mg=0 weights on the opposite queue so
                # the PE can start as soon as pair 0 lands.
                wb_pre = None
                if c == 0:
                    wb_pre = []
                zhi = [zpool.tile([P, 2, TC], FP8, tag=f"zhi{k}", name=f"zhi{k}")
                       for k in range(KP1)]
                zlo = [zpool.tile([P, 2, TC], FP8, tag=f"zlo{k}", name=f"zlo{k}")
                       for k in range(N1)]
                for kp in range(KP1):
                    xt = xpool.tile([P, 2, TC], F16, tag="xt")
                    nc.scalar.dma_start(xt[:],
                                        xT[kp * P:(kp + 1) * P, :, t0:t0 + TC])
                    if wb_pre is not None:
                        wt = wpool.tile([P, 2, MG * P], FP8, tag="wb",
                                        name=f"wbp{kp}")
                        wq = nc.sync if kp % 2 == 0 else nc.gpsimd
                        wq.dma_start(wt[:], Bd[kp * P:(kp + 1) * P, 0:WG])
                        wb_pre.append(wt)
                    for kk in range(2):
                        i = 2 * kp + kk
                        if c == 0 and (kp == 0 or kp >= N1):
                            nc.vector.tensor_scalar_mul(
                                zhi[kp][:, kk, :], xt[:, kk, :],
                                s0_t[:, i:i + 1])
                        else:
                            nc.scalar.activation(
                                zhi[kp][:, kk, :], xt[:, kk, :],
                                mybir.ActivationFunctionType.Copy,
                                scale=s0_t[:, i:i + 1])
                        if kp < N1:
                            nc.vector.scalar_tensor_tensor(
                                zlo[kp][:, kk, :], xt[:, kk, :],
                                s0_t[:, i:i + 1], zhi[kp][:, kk, :],
                                mult, sub)

                # stage 1: h1 = B @ z; stage 2: scale by s2/4, split
                h1hi = [hpool.tile([P, 2, TC], FP8, tag=f"h1hi{k}",
                                   name=f"h1hi{k}") for k in range(KP3)]
                h1lo = [hpool.tile([P, 2, TC], FP8, tag=f"h1lo{k}",
                                   name=f"h1lo{k}") for k in range(N3)]
                for mg in range(nM // MG):
                    pss = [pspool.tile([P, TC], F32, tag="ps", name="ps")
                           for _ in range(MG)]
                    for kp in range(KP1):
                        if wb_pre is not None and mg == 0:
                            wt = wb_pre[kp]
                        else:
                            wt = wpool.tile([P, 2, MG * P], FP8, tag="wb")
                            q = nc.sync if kp % 2 == 0 else nc.gpsimd
                            q.dma_start(wt[:], Bd[kp * P:(kp + 1) * P,
                                                  mg * WG:(mg + 1) * WG])
                        hi_stop = kp == KP1 - 1 and N1 < KP1
                        lo_stop = kp == KP1 - 1 and N1 == KP1
                        for m_ in range(MG):
                            lhsT = wt[:, :, m_ * P:(m_ + 1) * P]
                            nc.tensor.matmul(pss[m_][:], lhsT, zhi[kp][:],
                                             start=(kp == 0), stop=hi_stop,
                                             perf_mode=dr)
                            if kp < N1:
                                nc.tensor.matmul(pss[m_][:], lhsT, zlo[kp][:],
                                                 start=False, stop=lo_stop,
                                                 perf_mode=dr)
                    for m_ in range(MG):
                        m = mg * MG + m_
                        mp, sl = m // 2, m % 2
                        nc.scalar.activation(
                            h1hi[mp][:, sl, :], pss[m_][:],
                            mybir.ActivationFunctionType.Copy,
                            scale=s2_t[:, m:m + 1])
                        if mp < N3:
                            nc.vector.scalar_tensor_tensor(
                                h1lo[mp][:, sl, :], pss[m_][:],
                                s2_t[:, m:m + 1], h1hi[mp][:, sl, :],
                                mult, sub)

                # stage 3: h2 = A @ h1s; stage 4: y = h2*s4 + bias
                for og in range(nO // MG):
                    pso = [pspool.tile([P, TC], F32, tag="ps", name="ps")
                           for _ in range(MG)]
                    for kp in range(KP3):
                        wt2 = wpool.tile([P, 2, MG * P], FP8, tag="wa")
                        q = nc.sync if kp % 2 == 0 else nc.gpsimd
                        q.dma_start(wt2[:], Ad[kp * P:(kp + 1) * P,
                                               og * WG:(og + 1) * WG])
                        hi_stop = kp == KP3 - 1 and N3 < KP3
                        lo_stop = kp == KP3 - 1 and N3 == KP3
                        for o_ in range(MG):
                            lhsT = wt2[:, :, o_ * P:(o_ + 1) * P]
                            nc.tensor.matmul(pso[o_][:], lhsT, h1hi[kp][:],
                                             start=(kp == 0), stop=hi_stop,
                                             perf_mode=dr)
                            if kp < N3:
                                nc.tensor.matmul(pso[o_][:], lhsT, h1lo[kp][:],
                                                 start=False, stop=lo_stop,
                                                 perf_mode=dr)
                    for o_ in range(MG):
                        o = og * MG + o_
                        yt = ypool.tile([P, TC], F32, tag="yt")
                        nc.vector.tensor_scalar(
                            yt[:], pso[o_][:], s4_t[:, o:o + 1],
                            bi_t[:, o:o + 1], mult, add)
                        yq = nc.scalar if o_ % 2 == 0 else nc.sync
                        yq.dma_start(yT[o * P:(o + 1) * P, t0:t0 + TC],
                                     yt[:])

    nc.compile()
    return nc


_NC_CACHE = None


def _get_nc():
    global _NC_CACHE
    if _NC_CACHE is None:
        _NC_CACHE = _build_nc()
    return _NC_CACHE


def _col_major(v):
    return np.ascontiguousarray(
        np.asarray(v, dtype=np.float32).reshape(-1, P).T)


def _pair_layout(WT, mg_tiles):
    """[K, M] -> [K/2, 2*M] DoubleRow layout: out[kp*128+p, mg*WG + kk*MG*128
    + m] = WT[kp*256 + kk*128 + p, mg*MG*128 + m], cast to fp8 e4m3."""
    K, M = WT.shape
    w = WT.reshape(K // 256, 2, P, mg_tiles, MG * P).transpose(0, 2, 3, 1, 4)
    return np.ascontiguousarray(w).astype(E4M3).reshape(K // 2, 2 * M)


def make_in_maps(x, scaling0, B, scaling2, A, scaling4, bias):
    x = np.asarray(x, dtype=np.float32)
    s0 = np.asarray(scaling0, dtype=np.float32)
    s2 = np.asarray(scaling2, dtype=np.float32)
    B = np.asarray(B, dtype=np.float32)
    A = np.asarray(A, dtype=np.float32)

    # magnitude-sorted contraction dims (descending scaling)
    p1 = np.argsort(-s0)
    p3 = np.argsort(-s2)

    xf = x.reshape(BATCH * SEQ, IN_D)[:, p1].astype(np.float16)
    Bd = _pair_layout(np.ascontiguousarray(B[np.ix_(p3, p1)].T),
                      MID_D // (MG * P))
    Ad = _pair_layout(np.ascontiguousarray(A[:, p3].T), OUT_D // (MG * P))
    sc = np.ascontiguousarray(np.concatenate(
        [_col_major(v) for v in
         (s0[p1], s2[p3] * SDOWN,
          np.asarray(scaling4, dtype=np.float32) / SDOWN, bias)], axis=1))

    in_maps = []
    for c in range(N_CORES):
        xs = xf[c * T_CORE:(c + 1) * T_CORE]
        # [t, kp, kk, p] -> [kp, p, kk, t]
        xsw = np.ascontiguousarray(
            xs.reshape(T_CORE, KP1, 2, P).transpose(1, 3, 2, 0))
        in_maps.append({
            "xT": xsw.reshape(IN_D // 2, 2, T_CORE),
            "Bd": Bd, "Ad": Ad, "sc": sc,
        })
    return in_maps


def kernel(x, scaling0, B, scaling2, A, scaling4, bias):
    # The profile hook isn't available in every environment; force the
    # plain execution path.
    os.environ.setdefault("BASS_NEVER_TRACE", "1")

    in_maps = make_in_maps(x, scaling0, B, scaling2, A, scaling4, bias)
    nc = _get_nc()
    res = bass_utils.run_bass_kernel_spmd(
        nc, in_maps, core_ids=list(range(N_CORES)))

    y = np.empty((BATCH * SEQ, OUT_D), dtype=np.float32)
    for c in range(N_CORES):
        y[c * T_CORE:(c + 1) * T_CORE] = res.results[c]["yT"].T
    return y.reshape(BATCH, SEQ, OUT_D)
